# revision 29
# baseline (speedup 1.0000x reference)
"""Multi-head attention (B=2, Q=K=2048, H=16, D=V=64) on 8 Trainium2 cores.

Sharding: batch x heads. Core c handles batch b = c//4 and heads
[4*(c%4), 4*(c%4)+4) -- 4 (b,h) "pairs" per core, no cross-core comm.

Key optimizations over the naive dense version (~199us -> ~75us):

1. Host-side mask compaction: softmax with key masking only involves the
   unmasked keys (~half of 2048). K/V are gathered to the unmasked set
   on the host (pure layout: gather/pad/cast), padded to a multiple of
   128 with zero keys + zero V''-rows, so padded slots contribute
   exactly 0 to both the numerator and denominator. KCp ~ 8 chunks
   instead of 16 -- ~2x less matmul and exp work. The program is built
   per KCp and cached, so any mask density works.

2. The PE runs a PURE matmul stream and reaches its 2.4 GHz DVFS
   p-state (it throttles to 1.2 GHz unless busy continuously for ~3us).
   mm2 trails mm1 by LAG windows so every PE dependency has multi-us
   slack; score matmuls are zero-padded to a 128-row contraction so mm1
   and mm2 share one PE tile config (no per-switch reconfig bubble).

3. mm2 computes O directly in q-major orientation: the exp'd score
   subtile [128k x 128q] is the STATIONARY operand and V'' the moving
   one, so no output transpose is ever needed. All four q-subtile
   accumulation chains share one PSUM bank: only the first matmul
   carries start=True -- it marks the whole 2KB zero-region pending, so
   the other chains' first writes land on pending-zero bytes and
   overwrite cleanly (PSUM zero-region granularity is one bank).

4. exp is split across TWO engines. The ACT engine (1 elem/cycle at
   1.2 GHz, the former sole bottleneck at ~84% busy) takes 4 of every
   5 windows; the DVE takes the rest via two custom microcode ops
   registered through dve_ops' documented extension list:
   exp(s/8) = [p(s)]^16 with p(s) = 1 + s(B1 + s(B2 + s B3)) ~
   exp(s/128). Pass 1 (EXP_P3S2_ANT) is a degree-3 Horner plus two
   in-op squarings -- exactly the v3 8-ALU-stage budget; pass 2
   (SQSQ_ANT) squares twice more. fp32 intermediates; exp rel err
   <= 4e-3 at the 6-sigma tail. (TRN2 DVE has no exp/pow/shift ALU
   ops, so Schraudolph-style bit tricks are impossible; the
   squaring-chain is the only shift-free route.)

5. Normalization is 3 short DVE ops straight off PSUM (+eps add,
   reciprocal of the denominator column, per-partition scale) and a
   bf16 output DMA. V'' = [V | ones | zero-pad to 80] provides the
   denominator column for free in mm2's 80-wide moving operand.

Engine balance per block (measured): PE 70%, ACT 79%, DVE 72%; the
remaining ~24us are the framework's fixed preamble/epilogue plus the
input-DMA ramp.

Device algorithm per (b,h) pair, per 512-wide q-block:
  for each window (2 k-chunks of 128):
    S^T[k,q] = (K-chunk d,k)^T @ (Q^T d,q)    TensorE (bf16, fp32 acc)
    E = exp(S/8)                               ACT or DVE -> SBUF bf16
    for j in 0..3: acc[q_j, :] += E_j^T @ V''  TensorE (q-major out)
  rec[q] = 1/(acc[q, 64] + eps); out[q, :] = acc[q, 0:64] * rec[q]
"""

import sys

import numpy as np

sys.path.insert(0, "/opt/trn_rl_repo")

import concourse.bacc as bacc
import concourse.mybir as mybir
import concourse.tile as tile
from concourse import dve_ops as _DO
from concourse.bass_utils import run_bass_kernel_spmd
from concourse.dve_spec import C0, C1, C2, One, Spec, Src0, _has_src1, lower, sq
from concourse.dve_uop import DveOpSpec


def _register_dve_op(name, spec):
    """Register a custom DVE op via the documented extension list
    (dve_ops.OPS); the uop table is generated per-NEFF so new rows in
    [17, 0x20) are self-contained in this process's compile."""
    for op in _DO.OPS:
        if op.name == name:
            return op
    row = max(_DO._SUB_OPCODE_FOR_NAME.values()) + 1
    assert row < 0x20
    _DO._SUB_OPCODE_FOR_NAME[name] = row
    shas = {}
    for ver in ("v3", "v4"):
        try:
            uops = lower(spec, ver=ver)
            shas[ver] = DveOpSpec(
                name=name, opcode=row, uops=uops, rd1_en=_has_src1(spec)
            ).sha(ver)
        except Exception:
            pass
    op = _DO.DveOp(name, spec, False, shas)
    _DO.OPS.append(op)
    _DO.CUSTOM_DVE_SPECS[name] = spec
    return op


# exp(s/8) = [p(s)]^16 with p(s) = 1 + s(B1 + s(B2 + s*B3)) ~ exp(s/128),
# fitted over s in [-48, 48] (scores ~ N(0, 64); 6 sigma). Pass 1 computes
# p(s)^4 (Horner + 2 in-op squarings = 8 ALU stages, the v3 limit); pass 2
# squares twice more. End-to-end exp rel err <= 3.9e-4 typical / 3.9e-3 at
# the extreme tail; intermediates stay fp32.
B1, B2, B3 = 0.007815092628341121, 3.076957821164759e-05, 7.79752377112387e-08
_OP_EXP = _register_dve_op(
    "EXP_P3S2_ANT",
    Spec(
        body=sq(sq(((Src0 * C2 + C1) * Src0 + C0) * Src0 + One)),
        reference=lambda in0, in1, s0, s1, imm2: (
            1.0 + in0.astype(np.float32) * (s0 + in0 * (s1 + in0 * imm2))
        )
        ** 4,
    ),
)
_OP_SQSQ = _register_dve_op(
    "SQSQ_ANT",
    Spec(
        body=sq(sq(Src0)),
        reference=lambda in0, in1, s0, s1, imm2: in0.astype(np.float32) ** 4,
    ),
)

N_CORES = 8
B, Q, K, H, D, V = 2, 2048, 2048, 16, 64, 64
PAIRS = 4            # (b,h) pairs per core
QBW = 512            # q-block width
QB = Q // QBW        # 4 q-blocks
VP = 80              # V'' columns: 64 V + 1 ones + 15 zero pad (multiple of 16)
G = 2                # k-chunks per exp window
LAG = 4              # mm2 trails mm1 by LAG windows

F32 = mybir.dt.float32
BF16 = mybir.dt.bfloat16
NP_BF16 = mybir.dt.np(BF16)

_cached = {}
LAST_RESULTS = None


def _build_program(kcp):
    nc = bacc.Bacc("TRN2", target_bir_lowering=False, debug=False, num_devices=N_CORES)

    qT = nc.dram_tensor("qT", [PAIRS, 64, Q], BF16, kind="ExternalInput").ap()
    kT = nc.dram_tensor("kT", [PAIRS, 64, kcp * 128], BF16, kind="ExternalInput").ap()
    vpp = nc.dram_tensor("vpp", [PAIRS, 128, kcp, VP], BF16, kind="ExternalInput").ap()
    # out: [pair, blk, 128 q-in-subtile, subtile j, V] bf16
    o = nc.dram_tensor("o", [PAIRS, QB, 128, QBW // 128, V], BF16, kind="ExternalOutput").ap()

    # window structure per (pair, blk): chunk lists; ~every 5th window's
    # exp runs on DVE (2-pass custom op) to offload the saturated ACT; the
    # last one sits near the end so ACT and DVE overlap in the tail
    sizes = [G] * (kcp // G) + ([kcp % G] if kcp % G else [])
    n_win_tot = len(sizes) * PAIRS * QB
    DVE_WIN_IDS = {i for i in range(n_win_tot) if i % 5 == 4 and i < n_win_tot - 9}
    DVE_WIN_IDS.add(n_win_tot - 2)
    windows = []  # (p, blk, chunks, on_dve, last_of_block)
    for p in range(PAIRS):
        for blk in range(QB):
            c0 = 0
            for w, sz in enumerate(sizes):
                windows.append(
                    (p, blk, list(range(c0, c0 + sz)),
                     len(windows) in DVE_WIN_IDS, w == len(sizes) - 1)
                )
                c0 += sz
    T = len(windows)

    with tile.TileContext(nc) as tc:
        with (
            tc.sbuf_pool(name="persist", bufs=1) as persist,
            tc.sbuf_pool(name="epool", bufs=7) as epool,
            tc.sbuf_pool(name="w1pool", bufs=3) as w1pool,
            tc.sbuf_pool(name="norm", bufs=2) as normp,
            tc.psum_pool(name="win", bufs=3) as winp,
            tc.psum_pool(name="accp", bufs=2) as accp,
        ):
            # ---------------- input prep (direct bf16 DMAs) ----------------
            qTb, kTb, vpb = [], [], []
            dma_eng = [nc.sync, nc.gpsimd, nc.sync, nc.gpsimd]
            pad_eng = [nc.gpsimd, nc.vector, nc.gpsimd, nc.vector]
            for p in range(PAIRS):
                qb = persist.tile([128, Q], BF16, tag=f"qTb{p}")
                pad_eng[p].memzero(qb[64:128, :])
                qTb.append(qb)
                kb = persist.tile([128, kcp * 128], BF16, tag=f"kTb{p}")
                pad_eng[(p + 1) % 4].memzero(kb[64:128, :])
                kTb.append(kb)
                vb = persist.tile([128, kcp, VP], BF16, tag=f"vpp{p}")
                vpb.append(vb)
            # pair 0's first-window operands first so compute starts ASAP
            nc.sync.dma_start(out=kTb[0][0:64, :], in_=kT[0])
            nc.sync.dma_start(out=qTb[0][0:64, 0:QBW], in_=qT[0][:, 0:QBW])
            nc.sync.dma_start(out=vpb[0], in_=vpp[0])
            nc.sync.dma_start(out=qTb[0][0:64, QBW:], in_=qT[0][:, QBW:])
            for p in range(1, PAIRS):
                eng = dma_eng[p]
                eng.dma_start(out=qTb[p][0:64, :], in_=qT[p])
                eng.dma_start(out=kTb[p][0:64, :], in_=kT[p])
                eng.dma_start(out=vpb[p], in_=vpp[p])

            # ---------------- main pipeline ----------------
            acc_of = {}   # (p, blk) -> acc tile
            e_of = {}     # t -> e tile
            win_of = {}   # t -> win tile

            def emit_mm1(t, i):
                p, blk, chunks, _, _ = windows[t]
                c = chunks[i]
                if i == 0:
                    win_of[t] = winp.tile([128, G, QBW], F32, tag="win", name=f"win{t}")
                nc.tensor.matmul(
                    win_of[t][:, i, :],
                    kTb[p][:, c * 128 : (c + 1) * 128],
                    qTb[p][:, blk * QBW : (blk + 1) * QBW],
                    start=True,
                    stop=True,
                )

            def emit_mm2(t, i):
                p, blk, chunks, _, _ = windows[t]
                c = chunks[i]
                if (p, blk) not in acc_of:
                    acc_of[(p, blk)] = accp.tile([128, QBW // 128, VP], F32, tag="acc", name=f"acc{p}_{blk}")
                for j in range(QBW // 128):
                    # one start per PSUM bank: it marks the whole 2KB zero
                    # region pending, so the other j-chains' first writes
                    # land on pending-zero bytes and overwrite cleanly
                    nc.tensor.matmul(
                        acc_of[(p, blk)][:, j, :],
                        e_of[t][:, i, j * 128 : (j + 1) * 128],
                        vpb[p][:, c, :],
                        start=(c == 0 and j == 0),
                        stop=(c == kcp - 1 and j == QBW // 128 - 1),
                        skip_group_check=True,
                    )

            def emit_exp(t):
                _, _, chunks, on_dve, _ = windows[t]
                n = len(chunks)
                e_of[t] = epool.tile([128, G, QBW], BF16, tag="e", name=f"e{t}")
                if on_dve:
                    w1 = w1pool.tile([128, G, QBW], F32, tag="w1", name=f"w1{t}")
                    nc.vector._custom_dve(
                        _OP_EXP, out=w1[:, :n, :], in0=win_of[t][:, :n, :],
                        s0=B1, s1=B2, imm2=B3,
                    )
                    nc.vector._custom_dve(
                        _OP_SQSQ, out=e_of[t][:, :n, :], in0=w1[:, :n, :],
                    )
                else:
                    nc.scalar.activation(
                        out=e_of[t][:, :n, :],
                        in_=win_of[t][:, :n, :],
                        func=mybir.ActivationFunctionType.Exp,
                        scale=0.125,
                    )

            def emit_norm(p, blk):
                # mm2 output is already q-major: reciprocal + scale off PSUM
                acc = acc_of.pop((p, blk))
                deps = normp.tile([128, QBW // 128], F32, tag="deps")
                nc.vector.tensor_scalar_add(out=deps, in0=acc[:, :, V], scalar1=1e-10)
                rec = normp.tile([128, QBW // 128], F32, tag="rec")
                nc.vector.reciprocal(out=rec, in_=deps)
                osb = normp.tile([128, QBW // 128, V], BF16, tag="osb")
                nc.vector.tensor_tensor(
                    out=osb,
                    in0=acc[:, :, 0:V],
                    in1=rec.unsqueeze(2).broadcast_to([128, QBW // 128, V]),
                    op=mybir.AluOpType.mult,
                )
                nc.sync.dma_start(out=o[p, blk], in_=osb)

            for t in range(T + LAG):
                n1 = len(windows[t][2]) if t < T else 0
                n2 = len(windows[t - LAG][2]) if t >= LAG else 0
                for i in range(max(n1, n2)):
                    if i < n1:
                        emit_mm1(t, i)
                    if i < n2:
                        emit_mm2(t - LAG, i)
                if t == 1:
                    # V'' isn't needed until mm2 starts (LAG windows in):
                    # dispatching these DMAs here keeps them off the pre-loop
                    # critical path that gates the first matmul
                    for p2 in range(PAIRS):
                        dma_eng[p2].dma_start(out=vpb[p2], in_=vpp[p2])
                if t < T:
                    emit_exp(t)
                if t >= LAG + 2 and windows[t - LAG - 2][4]:
                    emit_norm(windows[t - LAG - 2][0], windows[t - LAG - 2][1])
            for t in range(T + LAG, T + LAG + 2):
                if windows[t - LAG - 2][4]:
                    emit_norm(windows[t - LAG - 2][0], windows[t - LAG - 2][1])

    nc.compile()
    return nc


def _get_program(kcp):
    if kcp not in _cached:
        _cached[kcp] = _build_program(kcp)
    return _cached[kcp]


def _shard_inputs(queries, keys, values, key_mask, kcp):
    queries = np.asarray(queries, dtype=np.float32)
    keys = np.asarray(keys, dtype=np.float32)
    values = np.asarray(values, dtype=np.float32)
    key_mask = np.asarray(key_mask, dtype=np.int32)

    kpad = kcp * 128
    # per-batch compaction of the key axis
    kc = np.zeros((B, kpad, H, D), dtype=np.float32)
    vc = np.zeros((B, kpad, VP), dtype=np.float32)  # built per (b,h) below
    vcs = []
    for b in range(B):
        idx = np.nonzero(key_mask[b])[0]
        n = len(idx)
        kc[b, :n] = keys[b, idx]
        vb = np.zeros((kpad, H, VP), dtype=np.float32)
        vb[:n, :, 0:V] = values[b, idx]
        vb[:n, :, V] = 1.0
        vcs.append(vb)

    # [B, S, H, D] -> [B, H, D, S]; rows 64..127 of the device tiles are
    # zeroed on-device so mm1 shares the PE's 128-row tile config with mm2
    qT_full = np.ascontiguousarray(queries.transpose(0, 2, 3, 1)).astype(NP_BF16)
    kT_full = np.ascontiguousarray(kc.transpose(0, 2, 3, 1)).astype(NP_BF16)

    in_maps = []
    for core in range(N_CORES):
        b, h0 = core // 4, (core % 4) * 4
        # vpp: [pair, 128, kcp, VP]; key k = c*128 + r -> [r, c]
        vpp = (
            vcs[b][:, h0 : h0 + 4, :]
            .reshape(kcp, 128, 4, VP)
            .transpose(2, 1, 0, 3)
        )
        in_maps.append(
            {
                "qT": np.ascontiguousarray(qT_full[b, h0 : h0 + 4]),
                "kT": np.ascontiguousarray(kT_full[b, h0 : h0 + 4]),
                "vpp": np.ascontiguousarray(vpp).astype(NP_BF16),
            }
        )
    return in_maps


def kernel(queries, keys, values, key_mask):
    global LAST_RESULTS
    key_mask = np.asarray(key_mask, dtype=np.int32)
    count = int(key_mask.sum(axis=1).max())
    kcp = max((count + 127) // 128, 1)

    nc = _get_program(kcp)
    in_maps = _shard_inputs(queries, keys, values, key_mask, kcp)
    res = run_bass_kernel_spmd(nc, in_maps, list(range(N_CORES)))
    LAST_RESULTS = res

    out = np.empty((B, Q, H * V), dtype=np.float32)
    for core in range(N_CORES):
        b, h0 = core // 4, (core % 4) * 4
        # [PAIRS, QB, 128(r), 4(j), V] -> q = blk*512 + j*128 + r
        oc = (
            res.results[core]["o"]
            .astype(np.float32)
            .transpose(0, 1, 3, 2, 4)
            .reshape(PAIRS, Q, V)
        )
        for p in range(PAIRS):
            h = h0 + p
            out[b, :, h * V : (h + 1) * V] = oc[p]
    return out


# revision 30
# speedup vs baseline: 1.0935x; 1.0935x over previous
"""Multi-head attention (B=2, Q=K=2048, H=16, D=V=64) on 8 Trainium2 cores.

Sharding: batch x heads. Core c handles batch b = c//4 and heads
[4*(c%4), 4*(c%4)+4) -- 4 (b,h) "pairs" per core, no cross-core comm.

Key optimizations over the naive dense version (~199us -> ~75us):

1. Host-side mask compaction: softmax with key masking only involves the
   unmasked keys (~half of 2048). K/V are gathered to the unmasked set
   on the host (pure layout: gather/pad/cast), padded to a multiple of
   128 with zero keys + zero V''-rows, so padded slots contribute
   exactly 0 to both the numerator and denominator. KCp ~ 8 chunks
   instead of 16 -- ~2x less matmul and exp work. The program is built
   per KCp and cached, so any mask density works.

2. The PE runs a PURE matmul stream and reaches its 2.4 GHz DVFS
   p-state (it throttles to 1.2 GHz unless busy continuously for ~3us).
   mm2 trails mm1 by LAG windows so every PE dependency has multi-us
   slack; score matmuls are zero-padded to a 128-row contraction so mm1
   and mm2 share one PE tile config (no per-switch reconfig bubble).

3. mm2 computes O directly in q-major orientation: the exp'd score
   subtile [128k x 128q] is the STATIONARY operand and V'' the moving
   one, so no output transpose is ever needed. All four q-subtile
   accumulation chains share one PSUM bank: only the first matmul
   carries start=True -- it marks the whole 2KB zero-region pending, so
   the other chains' first writes land on pending-zero bytes and
   overwrite cleanly (PSUM zero-region granularity is one bank).

4. exp is split across TWO engines. The ACT engine (1 elem/cycle at
   1.2 GHz, the former sole bottleneck at ~84% busy) takes 4 of every
   5 windows; the DVE takes the rest via two custom microcode ops
   registered through dve_ops' documented extension list:
   exp(s/8) = [p(s)]^16 with p(s) = 1 + s(B1 + s(B2 + s B3)) ~
   exp(s/128). Pass 1 (EXP_P3S2_ANT) is a degree-3 Horner plus two
   in-op squarings -- exactly the v3 8-ALU-stage budget; pass 2
   (SQSQ_ANT) squares twice more. fp32 intermediates; exp rel err
   <= 4e-3 at the 6-sigma tail. (TRN2 DVE has no exp/pow/shift ALU
   ops, so Schraudolph-style bit tricks are impossible; the
   squaring-chain is the only shift-free route.)

5. Normalization is 3 short DVE ops straight off PSUM (+eps add,
   reciprocal of the denominator column, per-partition scale) and a
   bf16 output DMA. V'' = [V | ones | zero-pad to 80] provides the
   denominator column for free in mm2's 80-wide moving operand.

Engine balance per block (measured): PE 70%, ACT 79%, DVE 72%; the
remaining ~24us are the framework's fixed preamble/epilogue plus the
input-DMA ramp.

Device algorithm per (b,h) pair, per 512-wide q-block:
  for each window (2 k-chunks of 128):
    S^T[k,q] = (K-chunk d,k)^T @ (Q^T d,q)    TensorE (bf16, fp32 acc)
    E = exp(S/8)                               ACT or DVE -> SBUF bf16
    for j in 0..3: acc[q_j, :] += E_j^T @ V''  TensorE (q-major out)
  rec[q] = 1/(acc[q, 64] + eps); out[q, :] = acc[q, 0:64] * rec[q]
"""

import sys

import numpy as np

sys.path.insert(0, "/opt/trn_rl_repo")

import concourse.bacc as bacc
import concourse.mybir as mybir
import concourse.tile as tile
from concourse import dve_ops as _DO
from concourse.bass_utils import run_bass_kernel_spmd
from concourse.dve_spec import C0, C1, C2, One, Spec, Src0, _has_src1, lower, sq
from concourse.dve_uop import DveOpSpec


def _register_dve_op(name, spec):
    """Register a custom DVE op via the documented extension list
    (dve_ops.OPS); the uop table is generated per-NEFF so new rows in
    [17, 0x20) are self-contained in this process's compile."""
    for op in _DO.OPS:
        if op.name == name:
            return op
    row = max(_DO._SUB_OPCODE_FOR_NAME.values()) + 1
    assert row < 0x20
    _DO._SUB_OPCODE_FOR_NAME[name] = row
    shas = {}
    for ver in ("v3", "v4"):
        try:
            uops = lower(spec, ver=ver)
            shas[ver] = DveOpSpec(
                name=name, opcode=row, uops=uops, rd1_en=_has_src1(spec)
            ).sha(ver)
        except Exception:
            pass
    op = _DO.DveOp(name, spec, False, shas)
    _DO.OPS.append(op)
    _DO.CUSTOM_DVE_SPECS[name] = spec
    return op


# exp(s/8) = [p(s)]^16 with p(s) = 1 + s(B1 + s(B2 + s*B3)) ~ exp(s/128),
# fitted over s in [-48, 48] (scores ~ N(0, 64); 6 sigma). Pass 1 computes
# p(s)^4 (Horner + 2 in-op squarings = 8 ALU stages, the v3 limit); pass 2
# squares twice more. End-to-end exp rel err <= 3.9e-4 typical / 3.9e-3 at
# the extreme tail; intermediates stay fp32.
B1, B2, B3 = 0.007815092628341121, 3.076957821164759e-05, 7.79752377112387e-08
_OP_EXP = _register_dve_op(
    "EXP_P3S2_ANT",
    Spec(
        body=sq(sq(((Src0 * C2 + C1) * Src0 + C0) * Src0 + One)),
        reference=lambda in0, in1, s0, s1, imm2: (
            1.0 + in0.astype(np.float32) * (s0 + in0 * (s1 + in0 * imm2))
        )
        ** 4,
    ),
)
_OP_SQSQ = _register_dve_op(
    "SQSQ_ANT",
    Spec(
        body=sq(sq(Src0)),
        reference=lambda in0, in1, s0, s1, imm2: in0.astype(np.float32) ** 4,
    ),
)

N_CORES = 8
B, Q, K, H, D, V = 2, 2048, 2048, 16, 64, 64
PAIRS = 4            # (b,h) pairs per core
QBW = 512            # q-block width
QB = Q // QBW        # 4 q-blocks
VP = 80              # V'' columns: 64 V + 1 ones + 15 zero pad (multiple of 16)
G = 2                # k-chunks per exp window
LAG = 4              # mm2 trails mm1 by LAG windows

F32 = mybir.dt.float32
BF16 = mybir.dt.bfloat16
NP_BF16 = mybir.dt.np(BF16)

_cached = {}
LAST_RESULTS = None


def _build_program(kcp):
    nc = bacc.Bacc("TRN2", target_bir_lowering=False, debug=False, num_devices=N_CORES)

    qT = nc.dram_tensor("qT", [PAIRS, 64, Q], BF16, kind="ExternalInput").ap()
    kT = nc.dram_tensor("kT", [PAIRS, 64, kcp * 128], BF16, kind="ExternalInput").ap()
    vpp = nc.dram_tensor("vpp", [PAIRS, 128, kcp, VP], BF16, kind="ExternalInput").ap()
    # out: [pair, blk, 128 q-in-subtile, subtile j, V] bf16
    o = nc.dram_tensor("o", [PAIRS, QB, 128, QBW // 128, V], BF16, kind="ExternalOutput").ap()

    # window structure per (pair, blk): chunk lists; ~every 5th window's
    # exp runs on DVE (2-pass custom op) to offload the saturated ACT; the
    # last one sits near the end so ACT and DVE overlap in the tail
    sizes = [G] * (kcp // G) + ([kcp % G] if kcp % G else [])
    n_win_tot = len(sizes) * PAIRS * QB
    DVE_WIN_IDS = {i for i in range(n_win_tot) if i % 5 == 4 and i < n_win_tot - 9}
    DVE_WIN_IDS.add(n_win_tot - 2)
    windows = []  # (p, blk, chunks, on_dve, last_of_block)
    for p in range(PAIRS):
        for blk in range(QB):
            c0 = 0
            for w, sz in enumerate(sizes):
                windows.append(
                    (p, blk, list(range(c0, c0 + sz)),
                     len(windows) in DVE_WIN_IDS, w == len(sizes) - 1)
                )
                c0 += sz
    T = len(windows)

    with tile.TileContext(nc) as tc:
        with (
            tc.sbuf_pool(name="persist", bufs=1) as persist,
            tc.sbuf_pool(name="epool", bufs=7) as epool,
            tc.sbuf_pool(name="w1pool", bufs=3) as w1pool,
            tc.sbuf_pool(name="norm", bufs=2) as normp,
            tc.psum_pool(name="win", bufs=3) as winp,
            tc.psum_pool(name="accp", bufs=2) as accp,
        ):
            # ---------------- input prep (direct bf16 DMAs) ----------------
            qTb, kTb, vpb = [], [], []
            dma_eng = [nc.sync, nc.gpsimd, nc.sync, nc.gpsimd]
            pad_eng = [nc.gpsimd, nc.vector, nc.gpsimd, nc.vector]
            for p in range(PAIRS):
                qb = persist.tile([128, Q], BF16, tag=f"qTb{p}")
                pad_eng[p].memzero(qb[64:128, :])
                qTb.append(qb)
                kb = persist.tile([128, kcp * 128], BF16, tag=f"kTb{p}")
                pad_eng[(p + 1) % 4].memzero(kb[64:128, :])
                kTb.append(kb)
                vb = persist.tile([128, kcp, VP], BF16, tag=f"vpp{p}")
                vpb.append(vb)
            # pair 0's first-window operands first so compute starts ASAP
            nc.sync.dma_start(out=kTb[0][0:64, :], in_=kT[0])
            nc.sync.dma_start(out=qTb[0][0:64, 0:QBW], in_=qT[0][:, 0:QBW])
            nc.sync.dma_start(out=vpb[0], in_=vpp[0])
            nc.sync.dma_start(out=qTb[0][0:64, QBW:], in_=qT[0][:, QBW:])
            for p in range(1, PAIRS):
                eng = dma_eng[p]
                eng.dma_start(out=qTb[p][0:64, :], in_=qT[p])
                eng.dma_start(out=kTb[p][0:64, :], in_=kT[p])
                eng.dma_start(out=vpb[p], in_=vpp[p])

            # ---------------- main pipeline ----------------
            acc_of = {}   # (p, blk) -> acc tile
            e_of = {}     # t -> e tile
            win_of = {}   # t -> win tile

            def emit_mm1(t, i):
                p, blk, chunks, _, _ = windows[t]
                c = chunks[i]
                if i == 0:
                    win_of[t] = winp.tile([128, G, QBW], F32, tag="win", name=f"win{t}")
                nc.tensor.matmul(
                    win_of[t][:, i, :],
                    kTb[p][:, c * 128 : (c + 1) * 128],
                    qTb[p][:, blk * QBW : (blk + 1) * QBW],
                    start=True,
                    stop=True,
                )

            def emit_mm2(t, i):
                p, blk, chunks, _, _ = windows[t]
                c = chunks[i]
                if (p, blk) not in acc_of:
                    acc_of[(p, blk)] = accp.tile([128, QBW // 128, VP], F32, tag="acc", name=f"acc{p}_{blk}")
                for j in range(QBW // 128):
                    # one start per PSUM bank: it marks the whole 2KB zero
                    # region pending, so the other j-chains' first writes
                    # land on pending-zero bytes and overwrite cleanly
                    nc.tensor.matmul(
                        acc_of[(p, blk)][:, j, :],
                        e_of[t][:, i, j * 128 : (j + 1) * 128],
                        vpb[p][:, c, :],
                        start=(c == 0 and j == 0),
                        stop=(c == kcp - 1 and j == QBW // 128 - 1),
                        skip_group_check=True,
                    )

            def emit_exp(t):
                _, _, chunks, on_dve, _ = windows[t]
                n = len(chunks)
                e_of[t] = epool.tile([128, G, QBW], BF16, tag="e", name=f"e{t}")
                if on_dve:
                    w1 = w1pool.tile([128, G, QBW], F32, tag="w1", name=f"w1{t}")
                    nc.vector._custom_dve(
                        _OP_EXP, out=w1[:, :n, :], in0=win_of[t][:, :n, :],
                        s0=B1, s1=B2, imm2=B3,
                    )
                    nc.vector._custom_dve(
                        _OP_SQSQ, out=e_of[t][:, :n, :], in0=w1[:, :n, :],
                    )
                else:
                    nc.scalar.activation(
                        out=e_of[t][:, :n, :],
                        in_=win_of[t][:, :n, :],
                        func=mybir.ActivationFunctionType.Exp,
                        scale=0.125,
                    )

            def emit_norm(p, blk):
                # mm2 output is already q-major: reciprocal + scale off PSUM
                acc = acc_of.pop((p, blk))
                deps = normp.tile([128, QBW // 128], F32, tag="deps")
                nc.vector.tensor_scalar_add(out=deps, in0=acc[:, :, V], scalar1=1e-10)
                rec = normp.tile([128, QBW // 128], F32, tag="rec")
                nc.vector.reciprocal(out=rec, in_=deps)
                osb = normp.tile([128, QBW // 128, V], BF16, tag="osb")
                nc.vector.tensor_tensor(
                    out=osb,
                    in0=acc[:, :, 0:V],
                    in1=rec.unsqueeze(2).broadcast_to([128, QBW // 128, V]),
                    op=mybir.AluOpType.mult,
                )
                nc.sync.dma_start(out=o[p, blk], in_=osb)

            for t in range(T + LAG):
                n1 = len(windows[t][2]) if t < T else 0
                n2 = len(windows[t - LAG][2]) if t >= LAG else 0
                for i in range(max(n1, n2)):
                    if i < n1:
                        emit_mm1(t, i)
                    if i < n2:
                        emit_mm2(t - LAG, i)
                if t < T:
                    emit_exp(t)
                if t >= LAG + 2 and windows[t - LAG - 2][4]:
                    emit_norm(windows[t - LAG - 2][0], windows[t - LAG - 2][1])
            for t in range(T + LAG, T + LAG + 2):
                if windows[t - LAG - 2][4]:
                    emit_norm(windows[t - LAG - 2][0], windows[t - LAG - 2][1])

    nc.compile()
    return nc


def _get_program(kcp):
    if kcp not in _cached:
        _cached[kcp] = _build_program(kcp)
    return _cached[kcp]


def _shard_inputs(queries, keys, values, key_mask, kcp):
    queries = np.asarray(queries, dtype=np.float32)
    keys = np.asarray(keys, dtype=np.float32)
    values = np.asarray(values, dtype=np.float32)
    key_mask = np.asarray(key_mask, dtype=np.int32)

    kpad = kcp * 128
    # per-batch compaction of the key axis
    kc = np.zeros((B, kpad, H, D), dtype=np.float32)
    vc = np.zeros((B, kpad, VP), dtype=np.float32)  # built per (b,h) below
    vcs = []
    for b in range(B):
        idx = np.nonzero(key_mask[b])[0]
        n = len(idx)
        kc[b, :n] = keys[b, idx]
        vb = np.zeros((kpad, H, VP), dtype=np.float32)
        vb[:n, :, 0:V] = values[b, idx]
        vb[:n, :, V] = 1.0
        vcs.append(vb)

    # [B, S, H, D] -> [B, H, D, S]; rows 64..127 of the device tiles are
    # zeroed on-device so mm1 shares the PE's 128-row tile config with mm2
    qT_full = np.ascontiguousarray(queries.transpose(0, 2, 3, 1)).astype(NP_BF16)
    kT_full = np.ascontiguousarray(kc.transpose(0, 2, 3, 1)).astype(NP_BF16)

    in_maps = []
    for core in range(N_CORES):
        b, h0 = core // 4, (core % 4) * 4
        # vpp: [pair, 128, kcp, VP]; key k = c*128 + r -> [r, c]
        vpp = (
            vcs[b][:, h0 : h0 + 4, :]
            .reshape(kcp, 128, 4, VP)
            .transpose(2, 1, 0, 3)
        )
        in_maps.append(
            {
                "qT": np.ascontiguousarray(qT_full[b, h0 : h0 + 4]),
                "kT": np.ascontiguousarray(kT_full[b, h0 : h0 + 4]),
                "vpp": np.ascontiguousarray(vpp).astype(NP_BF16),
            }
        )
    return in_maps


def kernel(queries, keys, values, key_mask):
    global LAST_RESULTS
    key_mask = np.asarray(key_mask, dtype=np.int32)
    count = int(key_mask.sum(axis=1).max())
    kcp = max((count + 127) // 128, 1)

    nc = _get_program(kcp)
    in_maps = _shard_inputs(queries, keys, values, key_mask, kcp)
    res = run_bass_kernel_spmd(nc, in_maps, list(range(N_CORES)))
    LAST_RESULTS = res

    out = np.empty((B, Q, H * V), dtype=np.float32)
    for core in range(N_CORES):
        b, h0 = core // 4, (core % 4) * 4
        # [PAIRS, QB, 128(r), 4(j), V] -> q = blk*512 + j*128 + r
        oc = (
            res.results[core]["o"]
            .astype(np.float32)
            .transpose(0, 1, 3, 2, 4)
            .reshape(PAIRS, Q, V)
        )
        for p in range(PAIRS):
            h = h0 + p
            out[b, :, h * V : (h + 1) * V] = oc[p]
    return out
